# revision 12
# baseline (speedup 1.0000x reference)
"""Trainium2 Bass kernel for nn_CVXPolicy_Quadcopter.

Computes, for each of B=500000 samples:
    p = MLP(concat([t, z]));  c = [(p6+p7+p8)/m, p9, p10, p11]
    ustar = -c * exp(-0.5 * W(||c||^2))   (W = Lambert W, via Newton)

v2 architecture (pure data parallel over 8 cores, 65536 samples/core):
  - host: inp_aug [14, B_pad] fp16 = [t; z^T; ones] (ones folds b1 into W1);
    W2 columns pre-combined + negated (W2cn, zero-padded to 32 cols); b2cn
    applied via the per-partition bias of the PSUM->SBUF copy.
  - loop over 128 tiles of 512 samples:
      mm1 (PE fp16): h_pre [100, 1536] per triad in PSUM
      tanh: ACT for most tiles; every POLY_MOD-th tile on DVE via an odd
            deg-7 polynomial in fp16 (offloads the ACT bottleneck)
      mm2 (PE): quad PSUM tile [128, 512], tile t at partition block 32*(t%4)
      copy (DVE, +bias): quad -> pk [128, 2048] staging, free = (b, q4, m)
            where n = 64*b + m, q4 = quad-in-chunk
      dump (1/chunk, SP): pk rows {32k+j} -> DRAM csc[g] = [j][k][2048]
      gather (1/quad): csc[g] -> c_all[32*q4:+32, 256g:+256]; the chunk
            layout is partition p = 32*q4 + 8*k + b, free = (j:4, m:64),
            so out column index is 8192*g + 64*p + m (a 3-dim DMA).
      dense per chunk: sq/s2/x + Lambert-W bits-init (Pool), then per pair
            of chunks one Newton step on v = w+1 (DVE + ACT exp),
            ne = exp((1-v)/2) (ACT), u = c*ne (Pool), out-DMA per chunk.
  - host: concat per-core u_t [4, 65536] -> [4, B], transpose to [B, 4].
"""

import sys

import numpy as np

for _p in ("/opt/trn_rl_repo", "/root/.axon_site/_ro/trn_rl_repo"):
    if _p not in sys.path:
        sys.path.append(_p)

_B = 500000
_NCORES = 8
_BLOC = 65536            # per-core padded batch: 128 tiles x 512
_BPAD = _BLOC * _NCORES
_NT = 512
_NTILES = 128
_MASS = 0.5
# w0 = C * (int_bits(1 + x) - B); 1 Newton iter reaches ~1e-3 overall
_LOG_B = 1064866805.0
_LOG_C = 6.197218803882235e-08
_NEWTON_ITERS = 1
_POLY_MOD = 10           # tiles with t % POLY_MOD == POLY_REM use DVE tanh
_POLY_REM = 9
# odd deg-7 fit of tanh on |x| <= 2.75: tanh(x) ~ x*(c0 + c1 r + c2 r^2
# + c3 r^3), r = x^2   (weighted LSQ; end-to-end L2 validated in numpy)
_PC = (0.9590737, -0.21994708, 0.03203638, -0.00178972)

_CACHE = {}


def _build_nc():
    import concourse.bacc as bacc
    import concourse.tile as tile
    from concourse import mybir

    f32 = mybir.dt.float32
    f16 = mybir.dt.float16
    i32 = mybir.dt.int32
    AF = mybir.ActivationFunctionType
    ALU = mybir.AluOpType

    nc = bacc.Bacc("TRN2", target_bir_lowering=False, debug=False,
                   num_devices=_NCORES)

    inp = nc.dram_tensor("inp", [14, _BLOC], f16, kind="ExternalInput")
    w1a = nc.dram_tensor("w1a", [14, 100], f16, kind="ExternalInput")
    w2cn = nc.dram_tensor("w2cn", [100, 32], f16, kind="ExternalInput")
    b2s = nc.dram_tensor("b2s", [128, 1], f32, kind="ExternalInput")
    out = nc.dram_tensor("out", [4, _BLOC], f32, kind="ExternalOutput")

    with tile.TileContext(nc) as tc:
        with (
            tc.tile_pool(name="consts", bufs=1) as consts,
            tc.tile_pool(name="inpp", bufs=4) as inp_pool,
            tc.tile_pool(name="hs", bufs=3) as h_pool,
            tc.tile_pool(name="pk", bufs=2) as pk_pool,
            tc.tile_pool(name="big", bufs=1) as big_pool,
            tc.tile_pool(name="sm", bufs=2) as sm_pool,
            tc.tile_pool(name="hp", bufs=2, space="PSUM") as hp_pool,
            tc.tile_pool(name="cps", bufs=2, space="PSUM") as c_pool,
            tc.tile_pool(name="dram", bufs=1, space="DRAM") as dram_pool,
        ):
            w1a_sb = consts.tile([14, 100], f16, tag="w1a")
            nc.sync.dma_start(w1a_sb, w1a[:])
            w2c_sb = consts.tile([100, 32], f16, tag="w2c")
            nc.sync.dma_start(w2c_sb, w2cn[:])
            b2_sb = consts.tile([128, 1], f32, tag="b2s")
            nc.sync.dma_start(b2_sb, b2s[:])
            half = consts.tile([128, 1], f32, tag="half")
            nc.vector.memset(half, 0.5)

            c_all = big_pool.tile([128, 2048], f32, tag="c_all")
            u_all = big_pool.tile([128, 2048], f32, tag="u_all")
            x_all = big_pool.tile([128, 512], f32, tag="x_all")
            v_all = big_pool.tile([128, 512], f32, tag="v_all")
            xe_all = big_pool.tile([128, 512], f32, tag="xe_all")
            ne_all = big_pool.tile([128, 512], f32, tag="ne_all")

            # DRAM chunk staging: csc[g][j2][k][F], F = 256*b + 64*q4 + m.
            # The dump writes pk verbatim (j2 covers all 32 rows per block,
            # only j2 < 4 is real); the gather reads the real rows only.
            csc = dram_pool.tile([8, 32, 4, 2048], f32, tag="csc")

            it_ring = {}
            state = {"hp": None, "h": None, "cp": None, "pk": None}

            def poly_tanh(hp_sl, h_sl):
                """DVE fp16 odd deg-7 tanh: h_sl = tanh(hp_sl)."""
                xc = sm_pool.tile([100, _NT], f16, tag="pxc")
                nc.vector.tensor_copy(xc, hp_sl)
                r = sm_pool.tile([100, _NT], f16, tag="pr")
                nc.vector.tensor_mul(r, xc, xc)
                u = sm_pool.tile([100, _NT], f16, tag="pu")
                nc.vector.tensor_scalar(u, r, _PC[3], _PC[2],
                                        op0=ALU.mult, op1=ALU.add)
                u2 = sm_pool.tile([100, _NT], f16, tag="pu2")
                nc.vector.tensor_mul(u2, u, r)
                nc.vector.tensor_scalar_add(u, u2, _PC[1])
                nc.vector.tensor_mul(u2, u, r)
                nc.vector.tensor_scalar_add(u, u2, _PC[0])
                nc.vector.tensor_mul(h_sl, u, xc)

            def chunk_dense(g, on_dve):
                """x and Lambert-W init for chunk g (c_all cols 256g..+256).

                Runs on Pool normally; on DVE for the tail chunks (shorter
                serial latency once the loop has drained).
                """
                eng = nc.vector if on_dve else nc.gpsimd
                cav = c_all[:, 256 * g:256 * (g + 1)]
                sq = sm_pool.tile([128, 256], f32, tag="dsq")
                eng.tensor_mul(sq, cav, cav)
                s2 = sm_pool.tile([128, 128], f32, tag="ds2")
                eng.tensor_add(s2, sq[:, 0:128], sq[:, 128:256])
                x_sl = x_all[:, 64 * g:64 * g + 64]
                eng.tensor_add(x_sl, s2[:, 0:64], s2[:, 64:128])
                y = sm_pool.tile([128, 64], f32, tag="dy")
                eng.tensor_scalar_add(y, x_sl, 1.0)
                fi = sm_pool.tile([128, 64], f32, tag="dfi")
                eng.tensor_copy(fi, y.bitcast(i32))
                eng.tensor_scalar(
                    v_all[:, 64 * g:64 * g + 64], fi, _LOG_C,
                    _LOG_B * _LOG_C - 1.0, op0=ALU.mult, op1=ALU.subtract)
                eng.tensor_scalar_mul(
                    xe_all[:, 64 * g:64 * g + 64], x_sl, float(np.e))

            def newton_and_out(gp, on_dve):
                """Newton + ne for pair gp; u and out-DMA per chunk."""
                eng = nc.vector if on_dve else nc.gpsimd
                sl = slice(128 * gp, 128 * gp + 128)
                v_sl = v_all[:, sl]
                xe_sl = xe_all[:, sl]
                for _ in range(_NEWTON_ITERS):
                    f = sm_pool.tile([128, 128], f32, tag="nf")
                    nc.scalar.activation(f, v_sl, AF.Exp, scale=-1.0)
                    rv = sm_pool.tile([128, 128], f32, tag="nrv")
                    nc.vector.reciprocal(rv, v_sl)
                    p = sm_pool.tile([128, 128], f32, tag="np")
                    nc.vector.scalar_tensor_tensor(
                        p, v_sl, 1.0, v_sl, op0=ALU.subtract, op1=ALU.mult)
                    tt_ = sm_pool.tile([128, 128], f32, tag="ntt")
                    nc.vector.tensor_mul(tt_, xe_sl, f)
                    num = sm_pool.tile([128, 128], f32, tag="nnum")
                    nc.vector.scalar_tensor_tensor(
                        num, p, 1.0, tt_, op0=ALU.add, op1=ALU.add)
                    nc.vector.tensor_mul(v_sl, num, rv)
                # ne = exp(-w/2) = exp(-0.5*v + 0.5)
                nc.scalar.activation(ne_all[:, sl], v_sl, AF.Exp,
                                     scale=-0.5, bias=half[:])
                for g in (2 * gp, 2 * gp + 1):
                    # u = c * ne, ne broadcast over the 4 comps j
                    ne_b = ne_all[:, 64 * g:64 * g + 64].unsqueeze(
                        1).broadcast_to((128, 4, 64))
                    cs = slice(256 * g, 256 * g + 256)
                    eng.tensor_mul(
                        u_all[:, cs].rearrange("p (j m) -> p j m", j=4),
                        c_all[:, cs].rearrange("p (j m) -> p j m", j=4),
                        ne_b)
                    # out columns 8192 g + 64 p + m, comp plane j
                    dst = out[0:4, 8192 * g:8192 * (g + 1)].rearrange(
                        "j (p m) -> p j m", p=128)
                    src = u_all[:, cs].rearrange("p (j m) -> p j m", j=4)
                    nc.sync.dma_start(dst, src)

            for t in range(_NTILES):
                if t % 8 == 0:
                    o = t // 8
                    it_new = inp_pool.tile([14, 4096], f16, tag="inp")
                    it_ring[o] = it_new
                    if o == 0:
                        nc.sync.dma_start(it_new[:, 0:2048], inp[:, 0:2048])
                        nc.sync.dma_start(it_new[:, 2048:4096],
                                          inp[:, 2048:4096])
                    else:
                        nc.sync.dma_start(it_new,
                                          inp[:, 4096 * o:4096 * (o + 1)])
                tri = t % 3 if t < 126 else t - 126
                if tri == 0:
                    ntr = min(3, _NTILES - t)
                    state["hp"] = hp_pool.tile([100, _NT * ntr], f32,
                                               tag="hp", name="hp")
                    state["h"] = h_pool.tile([100, _NT * ntr], f16,
                                             tag="h", name="h")
                hp_cur, h_cur = state["hp"], state["h"]
                nc.tensor.matmul(
                    hp_cur[:, _NT * tri:_NT * (tri + 1)],
                    lhsT=w1a_sb[:],
                    rhs=it_ring[t // 8][:, _NT * (t % 8):_NT * (t % 8 + 1)],
                    start=True, stop=True,
                )
                if t % 3 == 2 or t == _NTILES - 1:
                    ntr = tri + 1
                    t0 = t - tri
                    i = 0
                    while i < ntr:
                        if (t0 + i) % _POLY_MOD == _POLY_REM:
                            poly_tanh(hp_cur[:, _NT * i:_NT * (i + 1)],
                                      h_cur[:, _NT * i:_NT * (i + 1)])
                            i += 1
                        else:
                            i1 = i + 1
                            while (i1 < ntr and
                                   (t0 + i1) % _POLY_MOD != _POLY_REM):
                                i1 += 1
                            nc.scalar.activation(
                                h_cur[:, _NT * i:_NT * i1],
                                hp_cur[:, _NT * i:_NT * i1], AF.Tanh)
                            i = i1
                    for i in range(ntr):
                        ti = t0 + i
                        if ti % 4 == 0:
                            state["cp"] = c_pool.tile([128, _NT], f32,
                                                      tag="c", name="cp")
                        cp_cur = state["cp"]
                        nc.tensor.matmul(
                            cp_cur[32 * (ti % 4):32 * (ti % 4) + 32, :],
                            lhsT=w2c_sb[:],
                            rhs=h_cur[:, _NT * i:_NT * (i + 1)],
                            start=True, stop=True,
                            tile_position=(0, 32 * (ti % 4)),
                        )
                        if ti % 4 != 3:
                            continue
                        # end of quad: copy (+bias) into pk staging with
                        # free layout (b, q4, m)
                        q4 = (ti // 4) % 4
                        if q4 == 0:
                            state["pk"] = pk_pool.tile([128, 2048], f32,
                                                       tag="pk", name="pk")
                        pk_cur = state["pk"]
                        pk_dst = pk_cur[:].rearrange(
                            "p (b q4 m) -> p q4 b m", b=8, q4=4)[:, q4]
                        cp_v = cp_cur[:].rearrange("p (b m) -> p b m", b=8)
                        nc.vector.tensor_scalar_add(pk_dst, cp_v, b2_sb[:])
                        if ti % 16 != 15:
                            continue
                        # end of chunk: dump + 4 gathers + dense
                        g = ti // 16
                        dump_dst = csc[g].rearrange("j2 k F -> k j2 F")
                        nc.sync.dma_start(dump_dst, pk_cur[:])
                        for q4g in range(4):
                            gsrc = csc[g].rearrange(
                                "j2 k (b q4 m) -> q4 (k b) j2 m",
                                b=8, q4=4)[q4g][:, 0:4, :]
                            gdst = c_all[32 * q4g:32 * q4g + 32,
                                         256 * g:256 * (g + 1)].rearrange(
                                "p (j m) -> p j m", j=4)
                            eng = nc.gpsimd if q4g % 2 else nc.sync
                            eng.dma_start(gdst, gsrc)
                        tail = g >= 6
                        chunk_dense(g, on_dve=tail)
                        if g % 2 == 1:
                            newton_and_out(g // 2, on_dve=tail)

    nc.compile()
    return nc


def _get_nc():
    if "nc" not in _CACHE:
        _CACHE["nc"] = _build_nc()
    return _CACHE["nc"]


def _host_prep(z, t, W1, b1, W2, b2):
    f32 = np.float32
    z = np.asarray(z, f32)
    t = np.asarray(t, f32)
    W1 = np.asarray(W1, f32)
    b1 = np.asarray(b1, f32)
    W2 = np.asarray(W2, f32)
    b2 = np.asarray(b2, f32)

    f16 = np.float16
    inp_aug = np.zeros((14, _BPAD), f16)
    inp_aug[0, :_B] = t.astype(f16)
    inp_aug[1:13, :_B] = z.T.astype(f16)
    inp_aug[13, :] = 1.0

    W1a = np.concatenate([W1, b1[None, :]], axis=0).astype(f16)   # [14, 100]

    W2cn = np.zeros((100, 32), np.float16)
    W2cn[:, 0] = (-(W2[:, 6] + W2[:, 7] + W2[:, 8]) / f32(_MASS)).astype(
        np.float16)
    W2cn[:, 1] = -W2[:, 9].astype(np.float16)
    W2cn[:, 2] = -W2[:, 10].astype(np.float16)
    W2cn[:, 3] = -W2[:, 11].astype(np.float16)

    b2cn = np.array([-(b2[6] + b2[7] + b2[8]) / _MASS,
                     -b2[9], -b2[10], -b2[11]], f32)
    b2s = np.zeros((128, 1), f32)   # bias rows 32k+j <- b2cn[j]
    for k in range(4):
        b2s[32 * k:32 * k + 4, 0] = b2cn

    return inp_aug, W1a, W2cn, b2s


def kernel(z, t, W1, b1, W2, b2):
    from concourse.bass_utils import run_bass_kernel_spmd

    inp_aug, W1a, W2cn, b2s = _host_prep(z, t, W1, b1, W2, b2)
    nc = _get_nc()

    in_maps = []
    for c in range(_NCORES):
        in_maps.append({
            "inp": np.ascontiguousarray(
                inp_aug[:, _BLOC * c:_BLOC * (c + 1)]),
            "w1a": W1a,
            "w2cn": W2cn,
            "b2s": b2s,
        })

    res = run_bass_kernel_spmd(nc, in_maps, core_ids=list(range(_NCORES)))
    ut = np.concatenate([res.results[c]["out"] for c in range(_NCORES)],
                        axis=1)                                   # [4, BPAD]
    return np.ascontiguousarray(ut[:, :_B].T)                     # [B, 4]


# revision 79
# speedup vs baseline: 1.4032x; 1.4032x over previous
"""Trainium2 Bass kernel for nn_CVXPolicy_Quadcopter.

Computes, for each of B=500000 samples:
    p = MLP(concat([t, z]));  c = [(p6+p7+p8)/m, p9, p10, p11]
    ustar = -c * exp(-0.5 * W(||c||^2))   (W = Lambert W, via Newton)

v2 architecture (pure data parallel over 8 cores, 65536 samples/core):
  - host: inp_aug [14, B_pad] fp16 = [t; z^T; ones] (ones folds b1 into W1);
    W2 columns pre-combined + negated (W2cn, zero-padded to 32 cols); b2cn
    applied via the per-partition bias of the PSUM->SBUF copy.
  - loop over 128 tiles of 512 samples:
      mm1 (PE fp16): h_pre [100, 1536] per triad in PSUM
      tanh: ACT for most tiles; every POLY_MOD-th tile on DVE via an odd
            deg-7 polynomial in fp16 (offloads the ACT bottleneck)
      mm2 (PE): quad PSUM tile [128, 512], tile t at partition block 32*(t%4)
      copy (DVE, +bias): quad -> pk [128, 2048] staging, free = (b, q4, m)
            where n = 64*b + m, q4 = quad-in-chunk
      dump (1/chunk, SP): pk rows {32k+j} -> DRAM csc[g] = [j][k][2048]
      gather (1/quad): csc[g] -> c_all[32*q4:+32, 256g:+256]; the chunk
            layout is partition p = 32*q4 + 8*k + b, free = (j:4, m:64),
            so out column index is 8192*g + 64*p + m (a 3-dim DMA).
      dense per chunk: sq/s2/x + Lambert-W bits-init (Pool), then per pair
            of chunks one Newton step on v = w+1 (DVE + ACT exp),
            ne = exp((1-v)/2) (ACT), u = c*ne (Pool), out-DMA per chunk.
  - host: concat per-core u_t [4, 65536] -> [4, B], transpose to [B, 4].
"""

import sys

import numpy as np

for _p in ("/opt/trn_rl_repo", "/root/.axon_site/_ro/trn_rl_repo"):
    if _p not in sys.path:
        sys.path.append(_p)

_B = 500000
_NCORES = 8
_BLOC = 65536            # per-core padded batch: 128 tiles x 512
_BPAD = _BLOC * _NCORES
_NT = 512
_NTILES = 128
_MASS = 0.5
# w0 = C * (int_bits(1 + x) - B); 1 Newton iter reaches ~1e-3 overall
_LOG_B = 1064866805.0
_LOG_C = 6.197218803882235e-08
_NEWTON_ITERS = 1
# tiles with t % POLY_MOD == POLY_REM use the DVE polynomial tanh.
# DISABLED (MOD > 128): the DVE backlog holds the hp PSUM buffer via the
# poly's first read, stalling PE/ACT more than the ACT offload saves.
_POLY_MOD = 1000
_POLY_REM = 999
# odd deg-7 fit of tanh on |x| <= 2.75: tanh(x) ~ x*(c0 + c1 r + c2 r^2
# + c3 r^3), r = x^2   (weighted LSQ; end-to-end L2 validated in numpy)
_PC = (0.9590737, -0.21994708, 0.03203638, -0.00178972)

_CACHE = {}


def _build_nc():
    import concourse.bacc as bacc
    import concourse.tile as tile
    from concourse import mybir

    f32 = mybir.dt.float32
    f16 = mybir.dt.float16
    i32 = mybir.dt.int32
    AF = mybir.ActivationFunctionType
    ALU = mybir.AluOpType

    nc = bacc.Bacc("TRN2", target_bir_lowering=False, debug=False,
                   num_devices=_NCORES)

    inp = nc.dram_tensor("inp", [14, _BLOC], f16, kind="ExternalInput")
    w1a = nc.dram_tensor("w1a", [14, 100], f16, kind="ExternalInput")
    w2cn = nc.dram_tensor("w2cn", [100, 32], f16, kind="ExternalInput")
    b2s = nc.dram_tensor("b2s", [128, 1], f32, kind="ExternalInput")
    out = nc.dram_tensor("out", [4, _BLOC], f32, kind="ExternalOutput")

    with tile.TileContext(nc) as tc:
        with (
            tc.tile_pool(name="consts", bufs=1) as consts,
            tc.tile_pool(name="inpp", bufs=8) as inp_pool,
            tc.tile_pool(name="hs", bufs=3) as h_pool,
            tc.tile_pool(name="pk", bufs=2) as pk_pool,
            tc.tile_pool(name="big", bufs=1) as big_pool,
            tc.tile_pool(name="sm", bufs=2) as sm_pool,
            tc.tile_pool(name="hp", bufs=2, space="PSUM") as hp_pool,
            tc.tile_pool(name="cps", bufs=2, space="PSUM") as c_pool,
            tc.tile_pool(name="dram", bufs=1, space="DRAM") as dram_pool,
        ):
            w1a_sb = consts.tile([14, 100], f16, tag="w1a")
            nc.sync.dma_start(w1a_sb, w1a[:])
            w2c_sb = consts.tile([100, 32], f16, tag="w2c")
            nc.sync.dma_start(w2c_sb, w2cn[:])
            b2_sb = consts.tile([128, 1], f32, tag="b2s")
            nc.sync.dma_start(b2_sb, b2s[:])
            half = consts.tile([128, 1], f32, tag="half")
            nc.vector.memset(half, 0.5)

            c_all = big_pool.tile([128, 2048], f32, tag="c_all")
            u_all = big_pool.tile([128, 2048], f32, tag="u_all")
            x_all = big_pool.tile([128, 512], f32, tag="x_all")
            v_all = big_pool.tile([128, 512], f32, tag="v_all")
            xe_all = big_pool.tile([128, 512], f32, tag="xe_all")
            ne_all = big_pool.tile([128, 512], f32, tag="ne_all")

            # DRAM chunk staging: csc[g][j2][k][F], F = 256*b + 64*q4 + m,
            # one verbatim dump per chunk (j2 32 rows/block, real j2 < 4)
            csc = dram_pool.tile([7, 32, 4, 2048], f32, tag="csc")
            # tail chunk 7 stages per quad: csc7[q4][j][k][b*64+m]
            csc7 = dram_pool.tile([4, 4, 4, 512], f32, tag="csc7")

            it_ring = {}
            state = {"hp": None, "h": None, "cp": None, "pk": None}

            def poly_tanh(hp_sl, h_sl):
                """DVE fp16 odd deg-7 tanh: h_sl = tanh(hp_sl)."""
                xc = sm_pool.tile([100, _NT], f16, tag="pxc")
                nc.vector.tensor_copy(xc, hp_sl)
                r = sm_pool.tile([100, _NT], f16, tag="pr")
                nc.vector.tensor_mul(r, xc, xc)
                u = sm_pool.tile([100, _NT], f16, tag="pu")
                nc.vector.tensor_scalar(u, r, _PC[3], _PC[2],
                                        op0=ALU.mult, op1=ALU.add)
                u2 = sm_pool.tile([100, _NT], f16, tag="pu2")
                nc.vector.tensor_mul(u2, u, r)
                nc.vector.tensor_scalar_add(u, u2, _PC[1])
                nc.vector.tensor_mul(u2, u, r)
                nc.vector.tensor_scalar_add(u, u2, _PC[0])
                nc.vector.tensor_mul(h_sl, u, xc)

            def chunk_dense(g, on_dve):
                """x and Lambert-W init for chunk g (c_all cols 256g..+256).

                sq/s2/x on Pool normally (DVE for the tail chunks); the
                Lambert-W init always on DVE (same stream as the Newton
                consumers, so no cross-engine wait on v0).
                """
                eng = nc.vector if on_dve else nc.gpsimd
                cav = c_all[:, 256 * g:256 * (g + 1)]
                sq = sm_pool.tile([128, 256], f32, tag="dsq")
                eng.tensor_mul(sq, cav, cav)
                s2 = sm_pool.tile([128, 128], f32, tag="ds2")
                eng.tensor_add(s2, sq[:, 0:128], sq[:, 128:256])
                x_sl = x_all[:, 64 * g:64 * g + 64]
                eng.tensor_add(x_sl, s2[:, 0:64], s2[:, 64:128])
                y = sm_pool.tile([128, 64], f32, tag="dy")
                eng.tensor_scalar_add(y, x_sl, 1.0)
                fi = sm_pool.tile([128, 64], f32, tag="dfi")
                eng.tensor_copy(fi, y.bitcast(i32))
                eng.tensor_scalar(
                    v_all[:, 64 * g:64 * g + 64], fi, _LOG_C,
                    _LOG_B * _LOG_C - 1.0, op0=ALU.mult, op1=ALU.subtract)
                eng.tensor_scalar_mul(
                    xe_all[:, 64 * g:64 * g + 64], x_sl, float(np.e))

            def newton_and_out(g0, ng, on_dve):
                """Newton + ne for chunks [g0, g0+ng); u + out per chunk."""
                eng = nc.vector if on_dve else nc.gpsimd
                sl = slice(64 * g0, 64 * (g0 + ng))
                w = 64 * ng
                v_sl = v_all[:, sl]
                xe_sl = xe_all[:, sl]
                for _ in range(_NEWTON_ITERS):
                    f = sm_pool.tile([128, w], f32, tag="nf")
                    nc.scalar.activation(f, v_sl, AF.Exp, scale=-1.0)
                    rv = sm_pool.tile([128, w], f32, tag="nrv")
                    nc.vector.reciprocal(rv, v_sl)
                    p = sm_pool.tile([128, w], f32, tag="np")
                    nc.vector.scalar_tensor_tensor(
                        p, v_sl, 1.0, v_sl, op0=ALU.subtract, op1=ALU.mult)
                    tt_ = sm_pool.tile([128, w], f32, tag="ntt")
                    eng.tensor_mul(tt_, xe_sl, f)
                    num = sm_pool.tile([128, w], f32, tag="nnum")
                    nc.vector.scalar_tensor_tensor(
                        num, p, 1.0, tt_, op0=ALU.add, op1=ALU.add)
                    nc.vector.tensor_mul(v_sl, num, rv)
                # ne = exp(-w/2) = exp(-0.5*v + 0.5)
                nc.scalar.activation(ne_all[:, sl], v_sl, AF.Exp,
                                     scale=-0.5, bias=half[:])
                for g in range(g0, g0 + ng):
                    # u = c * ne, ne broadcast over the 4 comps j
                    ne_b = ne_all[:, 64 * g:64 * g + 64].unsqueeze(
                        1).broadcast_to((128, 4, 64))
                    cs = slice(256 * g, 256 * g + 256)
                    eng.tensor_mul(
                        u_all[:, cs].rearrange("p (j m) -> p j m", j=4),
                        c_all[:, cs].rearrange("p (j m) -> p j m", j=4),
                        ne_b)
                    # out columns 8192 g + 64 p + m, comp plane j
                    dst = out[0:4, 8192 * g:8192 * (g + 1)].rearrange(
                        "j (p m) -> p j m", p=128)
                    src = u_all[:, cs].rearrange("p (j m) -> p j m", j=4)
                    (nc.sync if on_dve else nc.gpsimd).dma_start(dst, src)

            def fetch_block(o):
                """Fetch an 8-tile input block [14, 4096]."""
                it_new = inp_pool.tile([14, 4096], f16, tag="inp",
                                       name="it")
                it_ring[o] = it_new
                if o == 0:
                    nc.sync.dma_start(it_new[:, 0:2048], inp[:, 0:2048])
                    nc.sync.dma_start(it_new[:, 2048:4096],
                                      inp[:, 2048:4096])
                else:
                    nc.sync.dma_start(it_new,
                                      inp[:, 4096 * o:4096 * (o + 1)])

            # triads: tiles [3T, 3T+3) (last triad has 2). mm1 is emitted
            # one triad AHEAD of tanh/mm2 so PE's in-order stream runs
            # mm1(T+1) during tanh(T) instead of blocking behind mm2(T);
            # ACT then runs tanh back-to-back (it is the bottleneck).
            triads = [(3 * T, min(3 * T + 3, _NTILES) - 3 * T)
                      for T in range(43)]

            def emit_mm1_triad(T):
                t0, ntr = triads[T]
                hp_new = hp_pool.tile([100, _NT * ntr], f32,
                                      tag="hp", name="hp")
                h_new = h_pool.tile([100, _NT * ntr], f16,
                                    tag="h", name="h")
                for i in range(ntr):
                    t = t0 + i
                    if t == 0:
                        fetch_block(0)
                        fetch_block(1)
                        fetch_block(2)
                        fetch_block(3)
                    elif t % 8 == 2 and t // 8 + 4 <= 15:
                        fetch_block(t // 8 + 4)
                    nc.tensor.matmul(
                        hp_new[:, _NT * i:_NT * (i + 1)],
                        lhsT=w1a_sb[:],
                        rhs=it_ring[t // 8][:, _NT * (t % 8):
                                            _NT * (t % 8 + 1)],
                        start=True, stop=True,
                    )
                return hp_new, h_new

            ring = {0: emit_mm1_triad(0)}
            pk_hist = {}

            def emit_gather(gd, q4g, eng):
                gsrc = csc[gd].rearrange(
                    "j2 k (b q4 m) -> q4 (k b) j2 m",
                    b=8, q4=4)[q4g][:, 0:4, :]
                gdst = c_all[32 * q4g:32 * q4g + 32,
                             256 * gd:256 * (gd + 1)].rearrange(
                    "p (j m) -> p j m", j=4)
                eng.dma_start(gdst, gsrc)
            for T in range(43):
                hp_cur, h_cur = ring.pop(T)
                t0, ntr = triads[T]
                if True:
                    i = 0
                    while i < ntr:
                        if (t0 + i) % _POLY_MOD == _POLY_REM:
                            poly_tanh(hp_cur[:, _NT * i:_NT * (i + 1)],
                                      h_cur[:, _NT * i:_NT * (i + 1)])
                            i += 1
                        else:
                            i1 = i + 1
                            while (i1 < ntr and
                                   (t0 + i1) % _POLY_MOD != _POLY_REM):
                                i1 += 1
                            nc.scalar.activation(
                                h_cur[:, _NT * i:_NT * i1],
                                hp_cur[:, _NT * i:_NT * i1], AF.Tanh)
                            i = i1
                    if T + 1 < 43:
                        ring[T + 1] = emit_mm1_triad(T + 1)
                    for i in range(ntr):
                        ti = t0 + i
                        if ti % 4 == 0:
                            state["cp"] = c_pool.tile([128, _NT], f32,
                                                      tag="c", name="cp")
                        cp_cur = state["cp"]
                        nc.tensor.matmul(
                            cp_cur[32 * (ti % 4):32 * (ti % 4) + 32, :],
                            lhsT=w2c_sb[:],
                            rhs=h_cur[:, _NT * i:_NT * (i + 1)],
                            start=True, stop=True,
                            tile_position=(0, 32 * (ti % 4)),
                        )
                        # schedule vB, all one chunk late with wide slack:
                        # dump at +3, all 4 gathers at +11, dense at +15,
                        # newton pairs at +7 of odd chunks (8+ tiles after
                        # their dense) so the ACT exp never waits
                        ph = ti % 16
                        gd = ti // 16 - 1
                        if ph == 11 and 0 <= gd <= 6:
                            nc.sync.dma_start(
                                csc[gd].rearrange("j2 k F -> k j2 F"),
                                pk_hist[gd][:])
                        gd2 = ti // 16 - 2
                        if ph == 3 and 0 <= gd2 <= 5:
                            emit_gather(gd2, 0, nc.sync)
                            emit_gather(gd2, 1, nc.gpsimd)
                            emit_gather(gd2, 2, nc.sync)
                            emit_gather(gd2, 3, nc.gpsimd)
                        if ti == 127:
                            emit_gather(6, 0, nc.sync)
                            emit_gather(6, 1, nc.gpsimd)
                            emit_gather(6, 2, nc.sync)
                            emit_gather(6, 3, nc.gpsimd)
                        if ph == 11 and 0 <= gd2 <= 5:
                            chunk_dense(gd2, on_dve=False)
                        if ph == 15 and ti >= 63 and ti // 16 % 2 == 1:
                            newton_and_out((ti // 16 - 3) // 2 * 2, 2,
                                           on_dve=False)
                        if ti % 4 != 3:
                            continue
                        # end of quad: copy (+bias) into pk staging
                        g = ti // 16
                        q4 = (ti // 4) % 4
                        if q4 == 0:
                            state["pk"] = pk_pool.tile([128, 2048], f32,
                                                       tag="pk", name="pk")
                            pk_hist[g] = state["pk"]
                        pk_cur = state["pk"]
                        if g == 7:
                            # last chunk: contiguous quad layout; dump +
                            # gather for each quad fire one quad LATE (the
                            # copy is then stale - no queue holds), last
                            # quad immediately; SP is idle here
                            pk_dst = pk_cur[:, 512 * q4:512 * (q4 + 1)]
                            nc.vector.tensor_scalar_add(pk_dst, cp_cur[:],
                                                        b2_sb[:])
                            tqi = (ti - 112) // 4
                            todo = [tqi - 1] if tqi >= 1 else []
                            if ti == 127:
                                todo.append(tqi)
                            for tj in todo:
                                q4c = tj % 4
                                src_pk = pk_hist[7][
                                    :, 512 * q4c:512 * (q4c + 1)]
                                for k in range(4):
                                    nc.sync.dma_start(
                                        csc7[q4c, :, k, :],
                                        src_pk[32 * k:32 * k + 4])
                                gsrc = csc7[q4c].rearrange(
                                    "j k (b m) -> (k b) j m", b=8)
                                gdst = c_all[
                                    32 * q4c:32 * q4c + 32,
                                    256 * 7:256 * 8].rearrange(
                                    "p (j m) -> p j m", j=4)
                                eng = nc.gpsimd if q4c % 2 else nc.sync
                                eng.dma_start(gdst, gsrc)
                        else:
                            # regular chunks: striped pk layout (b, q4, m)
                            pk_dst = pk_cur[:].rearrange(
                                "p (b q4 m) -> p q4 b m", b=8, q4=4)[:, q4]
                            cp_v = cp_cur[:].rearrange(
                                "p (b m) -> p b m", b=8)
                            nc.vector.tensor_scalar_add(pk_dst, cp_v,
                                                        b2_sb[:])


            # epilogue: last two chunks solo on DVE (short serial chains)
            chunk_dense(6, on_dve=True)
            chunk_dense(7, on_dve=True)
            newton_and_out(6, 1, on_dve=True)
            newton_and_out(7, 1, on_dve=True)

    nc.compile()
    return nc


def _get_nc():
    if "nc" not in _CACHE:
        _CACHE["nc"] = _build_nc()
    return _CACHE["nc"]


def _host_prep(z, t, W1, b1, W2, b2):
    f32 = np.float32
    z = np.asarray(z, f32)
    t = np.asarray(t, f32)
    W1 = np.asarray(W1, f32)
    b1 = np.asarray(b1, f32)
    W2 = np.asarray(W2, f32)
    b2 = np.asarray(b2, f32)

    f16 = np.float16
    inp_aug = np.zeros((14, _BPAD), f16)
    inp_aug[0, :_B] = t.astype(f16)
    inp_aug[1:13, :_B] = z.T.astype(f16)
    inp_aug[13, :] = 1.0

    W1a = np.concatenate([W1, b1[None, :]], axis=0).astype(f16)   # [14, 100]

    W2cn = np.zeros((100, 32), np.float16)
    W2cn[:, 0] = (-(W2[:, 6] + W2[:, 7] + W2[:, 8]) / f32(_MASS)).astype(
        np.float16)
    W2cn[:, 1] = -W2[:, 9].astype(np.float16)
    W2cn[:, 2] = -W2[:, 10].astype(np.float16)
    W2cn[:, 3] = -W2[:, 11].astype(np.float16)

    b2cn = np.array([-(b2[6] + b2[7] + b2[8]) / _MASS,
                     -b2[9], -b2[10], -b2[11]], f32)
    b2s = np.zeros((128, 1), f32)   # bias rows 32k+j <- b2cn[j]
    for k in range(4):
        b2s[32 * k:32 * k + 4, 0] = b2cn

    return inp_aug, W1a, W2cn, b2s


def kernel(z, t, W1, b1, W2, b2):
    from concourse.bass_utils import run_bass_kernel_spmd

    inp_aug, W1a, W2cn, b2s = _host_prep(z, t, W1, b1, W2, b2)
    nc = _get_nc()

    in_maps = []
    for c in range(_NCORES):
        in_maps.append({
            "inp": np.ascontiguousarray(
                inp_aug[:, _BLOC * c:_BLOC * (c + 1)]),
            "w1a": W1a,
            "w2cn": W2cn,
            "b2s": b2s,
        })

    res = run_bass_kernel_spmd(nc, in_maps, core_ids=list(range(_NCORES)))
    ut = np.concatenate([res.results[c]["out"] for c in range(_NCORES)],
                        axis=1)                                   # [4, BPAD]
    return np.ascontiguousarray(ut[:, :_B].T)                     # [B, 4]
